# revision 10
# baseline (speedup 1.0000x reference)
"""Trainium2 Bass kernel for nn_KV_TaskMemory.

Computation:
    w   = softmax(cos_sim(task_emb, K_memory))          # [16]
    rec0 = einsum('m,mab->ab', w, V0)                   # [1024, 1024]
    rec1 = einsum('m,mab->ab', w, V1)                   # [1024, 4096]

Strategy: shard V0/V1 along the row ('a') axis across 8 cores (128 rows
each -> fully contiguous per-slot DMA blocks). K_memory / task_emb are
replicated; every core computes the softmax weights redundantly, then
runs a DVE multiply-accumulate chain over the 16 memory slots. Pure
data-parallel; no collectives. The kernel is HBM-DMA bound
(~42 MB/core @ ~360 GB/s => ~117 us floor).
"""

import numpy as np
from contextlib import ExitStack

M = 16
KD = 1024
A = 1024
B0 = 1024
B1 = 4096
NCORES = 8
ROWS = A // NCORES  # 128
V1_CHUNK = 2048

_built = None


def _trace_body(ctx, tc, task, kmem, v0, v1, r0, r1, w_dram):
    import concourse.bass as bass
    from concourse import mybir

    f32 = mybir.dt.float32
    Alu = mybir.AluOpType
    nc = tc.nc

    wpool = ctx.enter_context(tc.tile_pool(name="wpool", bufs=1))
    v0pool = ctx.enter_context(tc.tile_pool(name="v0pool", bufs=6))
    v1pool = ctx.enter_context(tc.tile_pool(name="v1pool", bufs=6))
    accpool = ctx.enter_context(tc.tile_pool(name="accpool", bufs=1))

    # ---- softmax weights (redundant on every core) ----
    kt = wpool.tile([M, KD], f32)
    nc.sync.dma_start(kt[:], kmem[:])
    # broadcast task row to 16 partitions via stride-0 DRAM read
    te16 = wpool.tile([M, KD], f32)
    nc.sync.dma_start(te16[:], task.broadcast_to((M, KD)))

    junk0 = wpool.tile([M, KD], f32)
    junk1 = wpool.tile([M, KD], f32)
    junk2 = wpool.tile([M, KD], f32)
    dot = wpool.tile([M, 1], f32)
    k2 = wpool.tile([M, 1], f32)
    t2 = wpool.tile([M, 1], f32)
    # accum_out = sum over free dim of (in0 * in1)
    nc.vector.scalar_tensor_tensor(
        junk0[:], kt[:], 1.0, te16[:], op0=Alu.mult, op1=Alu.mult, accum_out=dot[:]
    )
    nc.vector.scalar_tensor_tensor(
        junk1[:], kt[:], 1.0, kt[:], op0=Alu.mult, op1=Alu.mult, accum_out=k2[:]
    )
    nc.vector.scalar_tensor_tensor(
        junk2[:], te16[:], 1.0, te16[:], op0=Alu.mult, op1=Alu.mult, accum_out=t2[:]
    )

    nrm2 = wpool.tile([M, 1], f32)
    nc.vector.tensor_mul(nrm2[:], k2[:], t2[:])
    nrm = wpool.tile([M, 1], f32)
    nc.scalar.sqrt(nrm[:], nrm2[:])
    nrmc = wpool.tile([M, 1], f32)
    nc.vector.tensor_scalar_max(nrmc[:], nrm[:], 1e-6)
    rcp = wpool.tile([M, 1], f32)
    nc.vector.reciprocal(rcp[:], nrmc[:])
    cos = wpool.tile([M, 1], f32)
    nc.vector.tensor_mul(cos[:], dot[:], rcp[:])

    # transpose cos [16,1] -> row [1,16] (32x32 block transpose), then
    # softmax along the free dim on a single partition.
    # cos in [-1, 1] -> exp safe without max subtraction.
    sq = wpool.tile([32, 32], f32)
    nc.vector.memset(sq[:], 0.0)
    nc.vector.tensor_copy(sq[0:M, 0:1], cos[:])
    sqT = wpool.tile([32, 32], f32)
    nc.vector.transpose(sqT[:], sq[:])
    erow = wpool.tile([1, M], f32)
    nc.scalar.activation(erow[:], sqT[0:1, 0:M], mybir.ActivationFunctionType.Exp)
    s = wpool.tile([1, 1], f32)
    nc.vector.tensor_reduce(s[:], erow[:], axis=mybir.AxisListType.X, op=Alu.add)
    rs = wpool.tile([1, 1], f32)
    nc.vector.reciprocal(rs[:], s[:])
    wrow = wpool.tile([1, M], f32)
    nc.vector.tensor_scalar_mul(wrow[:], erow[:], rs[0:1, 0:1])

    # broadcast w row to all 128 partitions: bounce through DRAM, then a
    # stride-0 DRAM read replicates it into every partition.
    nc.sync.dma_start(w_dram[:], wrow[:])
    wb = wpool.tile([128, M], f32)
    nc.sync.dma_start(wb[:], w_dram.broadcast_to((128, M)))

    # ---- V0 MAC chain ----
    acc0 = accpool.tile([ROWS, B0], f32)
    for m in range(M):
        t = v0pool.tile([ROWS, B0], f32, tag="v0t")
        nc.sync.dma_start(t[:], v0[m])
        if m == 0:
            nc.vector.tensor_scalar_mul(acc0[:], t[:], wb[:, 0:1])
        else:
            nc.vector.scalar_tensor_tensor(
                acc0[:], t[:], wb[:, m : m + 1], acc0[:], op0=Alu.mult, op1=Alu.add
            )
    nc.sync.dma_start(r0[:], acc0[:])

    # ---- V1 MAC chains (column-chunked) ----
    acc1 = accpool.tile([ROWS, B1], f32)
    for c in range(B1 // V1_CHUNK):
        sl = slice(c * V1_CHUNK, (c + 1) * V1_CHUNK)
        for m in range(M):
            t = v1pool.tile([ROWS, V1_CHUNK], f32, tag="v1t")
            nc.sync.dma_start(t[:], v1[m, :, sl])
            if m == 0:
                nc.vector.tensor_scalar_mul(acc1[:, sl], t[:], wb[:, 0:1])
            else:
                nc.vector.scalar_tensor_tensor(
                    acc1[:, sl], t[:], wb[:, m : m + 1], acc1[:, sl],
                    op0=Alu.mult, op1=Alu.add,
                )
    nc.sync.dma_start(r1[:], acc1[:])


def _build_program():
    global _built
    if _built is not None:
        return _built
    import concourse.bass as bass
    import concourse.tile as tile
    from concourse import bacc, mybir

    f32 = mybir.dt.float32
    nc = bacc.Bacc("TRN2", target_bir_lowering=False, debug=False, num_devices=NCORES)
    task = nc.dram_tensor("task_emb", [1, KD], f32, kind="ExternalInput").ap()
    kmem = nc.dram_tensor("K_memory", [M, KD], f32, kind="ExternalInput").ap()
    v0 = nc.dram_tensor("V0", [M, ROWS, B0], f32, kind="ExternalInput").ap()
    v1 = nc.dram_tensor("V1", [M, ROWS, B1], f32, kind="ExternalInput").ap()
    r0 = nc.dram_tensor("rec0", [ROWS, B0], f32, kind="ExternalOutput").ap()
    r1 = nc.dram_tensor("rec1", [ROWS, B1], f32, kind="ExternalOutput").ap()
    w_dram = nc.dram_tensor("w_scratch", [1, M], f32).ap()

    with tile.TileContext(nc) as tc:
        with ExitStack() as ctx:
            _trace_body(ctx, tc, task, kmem, v0, v1, r0, r1, w_dram)
    nc.compile()
    _built = nc
    return nc


# test.py can flip this to get an NTFF-profiled run; exec_time lands in LAST_PROFILE.
TRACE = False
LAST_PROFILE = None


def kernel(task_emb, K_memory, V0, V1):
    global LAST_PROFILE
    from concourse.bass_utils import run_bass_kernel_spmd

    nc = _build_program()
    task_emb = np.ascontiguousarray(task_emb, dtype=np.float32)
    K_memory = np.ascontiguousarray(K_memory, dtype=np.float32)
    V0 = np.asarray(V0, dtype=np.float32)
    V1 = np.asarray(V1, dtype=np.float32)

    in_maps = []
    for c in range(NCORES):
        sl = slice(c * ROWS, (c + 1) * ROWS)
        in_maps.append(
            {
                "task_emb": task_emb,
                "K_memory": K_memory,
                "V0": np.ascontiguousarray(V0[:, sl, :]),
                "V1": np.ascontiguousarray(V1[:, sl, :]),
            }
        )
    out = run_bass_kernel_spmd(nc, in_maps, list(range(NCORES)), trace=TRACE)
    LAST_PROFILE = out
    res = out.results
    rec0 = np.concatenate([res[c]["rec0"] for c in range(NCORES)], axis=0)
    rec1 = np.concatenate([res[c]["rec1"] for c in range(NCORES)], axis=0)
    return rec0, rec1
